# revision 1
# baseline (speedup 1.0000x reference)
"""Trainium2 Bass kernel for nn_DeepModel_70703751626759 (deep-BSDE forward sim).

v3: wide concurrent matmul waves on rotated 32x32 PE tile positions;
dw broadcast moved off the PE (host-replicated, DMA-streamed); b3 folded
into pzu via a K=1 ones matmul; loss from bf16 SBUF at 2x DVE rate; SCA
on GpSimd off the critical path.  State stays fp32 in a persistent PSUM
accumulator; increments flow through bf16.

Data-parallel over 8 NeuronCores: 32768 samples -> 4096/core -> 8 slots
(p, q) of 512 samples; slot (p, q) state at partitions [32p, 32p+32) x
free [512q, 512q+512).  Slot content: XY = [X(16); Y(16)], ZU = [u(8);
Zv(16); dH(8)].  Per step: H1 slot (p,q) lives at group f1(p,q), H2 at
f2(p,q), ZU at LZ(p,q) so every matmul wave lands on distinct PE
sub-arrays.
"""

import sys
import numpy as np

if "/opt/trn_rl_repo" not in sys.path:
    sys.path.insert(0, "/opt/trn_rl_repo")

N = 16
M = 8
T = 50
DT = 0.01
GAMMA = 0.1
SIGMA = 0.2
TAU = 0.5
H = 10
BATCH = 32768
NCORES = 8
CB = BATCH // NCORES
BK = 512

F32 = np.float32
try:
    import ml_dtypes
    BF16 = ml_dtypes.bfloat16
except ImportError:          # pragma: no cover
    BF16 = np.float32

# bf16 consts image (ckb) column offsets
K_W1 = 0
K_W2 = 32
K_W3 = 64
K_WZY = 96
K_WDXY = 128
K_WDZU = 160
K_WDSA = 192
K_WDSB = 224
K_WE = 256
K_WY1 = 288
K_WY2 = 320
K_WY3 = 352
K_SGR = 384      # 2 rows at 32g / 32g+1
K_WYP = 416
K_COLS = 448

# fp32 consts image (ckf) column offsets
C_WYP = 0
C_B1 = 32          # 64 cols (t = 0..T-1)
C_B2 = 96
C_BY1 = 97
C_BY2 = 98
C_COLS = 128


def _ct(t):
    w = 1.0 if (t == 0 or t == T - 1) else 2.0
    return 0.5 * DT * w * TAU * TAU


# slot location maps (partition group of each intermediate, per (p, q))
# ZU must live at group p: off-diagonal accumulation into the long-lived
# PST PSUM group crashes the hardware (empirically), so the state-update
# wave stays on diagonal tile positions.
def _f1(p, q):
    return (p + 1) % 4 if q == 0 else (p + 2) % 4


def _f2(p, q):
    return (p + 3) % 4


def _lz(p, q):
    return p


def _weight_blocks(inp):
    A = np.asarray(inp["A"], F32)
    Bm = np.asarray(inp["Bmat"], F32)
    C = np.asarray(inp["Cmat"], F32)
    D = np.asarray(inp["Dmat"], F32)
    ZW1 = np.asarray(inp["Z_W1"], F32)
    ZW2 = np.asarray(inp["Z_W2"], F32)
    ZW3 = np.asarray(inp["Z_W3"], F32)
    PW1 = np.asarray(inp["phi_W1"], F32)
    PW2 = np.asarray(inp["phi_W2"], F32)
    PW3 = np.asarray(inp["phi_W3"], F32)
    YW1 = np.asarray(inp["Y0_W1"], F32)
    YW2 = np.asarray(inp["Y0_W2"], F32)
    YW3 = np.asarray(inp["Y0_W3"], F32)
    I16 = np.eye(16, dtype=F32)

    def blk():
        return np.zeros((32, 32), F32)

    W1 = blk()
    W1[0:16, 0:10] = ZW1[1:, :]
    W1[0:16, 10:20] = PW1[1:, :]
    W2 = blk()
    W2[0:10, 0:10] = ZW2
    W2[10:20, 10:20] = PW2
    # ZU rows: u(0:8) Zv(8:24) dH(24:32)
    W3 = blk()
    W3[10:20, 0:8] = PW3
    W3[0:10, 8:24] = ZW3
    W3[0:10, 24:32] = ZW3 @ D
    W3[10:20, 24:32] = PW3
    WZY = blk()
    WZY[16:32, 24:32] = Bm
    WDXY = blk()
    WDXY[0:16, 0:16] = DT * A.T
    WDXY[0:16, 16:32] = -DT * I16
    WDXY[16:32, 16:32] = -DT * A
    WDZU = blk()
    WDZU[0:8, 0:16] = DT * Bm.T
    WDZU[8:24, 16:32] = -DT * C
    WDSA = blk()
    WDSA[0:16, 0:16] = C.T
    WDSB = blk()
    WDSB[0:8, 0:16] = D.T
    WDSB[8:24, 16:32] = I16
    WE = blk()
    WE[0:16, 0:16] = -I16
    WE[16:32, 0:16] = I16
    WY1 = blk()
    WY1[0:16, 0:10] = YW1
    WY2 = blk()
    WY2[0:10, 0:10] = YW2
    WY3 = blk()
    WY3[0:10, 16:32] = YW3
    WYP = np.eye(32, dtype=F32)
    SGR = np.zeros((2, 32), F32)      # K=2 lhsT: [dw-row; ones-row]
    SGR[0, 0:16] = SIGMA
    SGR[1, 0:16] = GAMMA * DT
    b3 = np.concatenate([np.asarray(inp["phi_b3"], F32),
                         np.asarray(inp["Z_b3"], F32),
                         np.asarray(inp["phi_b3"], F32)
                         + D.T @ np.asarray(inp["Z_b3"], F32)])
    by3 = np.zeros(32, F32)
    by3[16:32] = np.asarray(inp["Y0_b3"], F32)
    return dict(W1=W1, W2=W2, W3=W3, WZY=WZY, WDXY=WDXY, WDZU=WDZU,
                WDSA=WDSA, WDSB=WDSB, WE=WE, WY1=WY1, WY2=WY2, WY3=WY3,
                WYP=WYP, SGR=SGR, b3=b3, by3=by3)


def pack_weights_bf16(inp):
    wb = _weight_blocks(inp)
    img = np.zeros((128, K_COLS), F32)
    reps = [(K_W1, "W1"), (K_W2, "W2"), (K_W3, "W3"), (K_WZY, "WZY"),
            (K_WDXY, "WDXY"), (K_WDZU, "WDZU"), (K_WDSA, "WDSA"),
            (K_WDSB, "WDSB"), (K_WE, "WE"), (K_WY1, "WY1"),
            (K_WY2, "WY2"), (K_WY3, "WY3")]
    for g in range(4):
        r = 32 * g
        for off, name in reps:
            img[r: r + 32, off: off + 32] = wb[name]
        img[r: r + 2, K_SGR: K_SGR + 32] = wb["SGR"]
        img[r: r + 32, K_WYP: K_WYP + 32] = wb["WYP"]
    return img.astype(BF16)


def pack_weights_f32(inp):
    wb = _weight_blocks(inp)
    Zb1 = np.asarray(inp["Z_b1"], F32)
    Pb1 = np.asarray(inp["phi_b1"], F32)
    Zb2 = np.asarray(inp["Z_b2"], F32)
    Pb2 = np.asarray(inp["phi_b2"], F32)
    Yb1 = np.asarray(inp["Y0_b1"], F32)
    Yb2 = np.asarray(inp["Y0_b2"], F32)
    ZW1 = np.asarray(inp["Z_W1"], F32)
    PW1 = np.asarray(inp["phi_W1"], F32)
    img = np.zeros((128, C_COLS), F32)
    b2 = np.concatenate([Zb2, Pb2])
    for t in range(T):
        tv = F32(t * DT)
        b = np.concatenate([Zb1 + tv * ZW1[0, :], Pb1 + tv * PW1[0, :]])
        for g in range(4):
            img[32 * g: 32 * g + 20, C_B1 + t] = b
    for g in range(4):
        r = 32 * g
        img[r: r + 32, C_WYP: C_WYP + 32] = wb["WYP"]
        img[r: r + 20, C_B2] = b2
        img[r: r + 10, C_BY1] = Yb1
        img[r: r + 10, C_BY2] = Yb2
    return img


def pack_x0(X0, core, by3):
    out = np.zeros((128, 1024), F32)
    base = core * CB
    for p in range(4):
        for q in range(2):
            j = p + 4 * q
            out[32 * p: 32 * p + 16, 512 * q: 512 * (q + 1)] = \
                X0[base + BK * j: base + BK * (j + 1), :].T
        out[32 * p + 16: 32 * p + 32, :] = by3[16:32, None]
    return np.ascontiguousarray(out.astype(BF16))


def pack_b3t(b3):
    out = np.zeros((128, 1024), F32)
    for p in range(4):
        out[32 * p: 32 * p + 32, :] = b3[:, None]
    return np.ascontiguousarray(out.astype(BF16))


def pack_dw_rep(dw, core):
    """Host-replicated dw broadcast tiles.

    dwa[t]: group g cols 512q = dw(slot (g, q)) replicated on all 32 rows.
    dws[t]: rows (2g, 2g+1) = [dw(slot (g, q)); ones] for cols 512q
            (SGR rhs pairs, landed at device partitions 32g..32g+2).
    Returned flattened to [128, T*1024] / [8, T*1024].
    """
    base = core * CB
    d = np.asarray(dw, F32)[:, base: base + CB, 0]      # [T, CB]
    blk = d.reshape(T, 8, BK)                           # slot j = p + 4q
    dwa = np.empty((T, 128, 1024), F32)
    dws = np.empty((T, 8, 1024), F32)
    for g in range(4):
        r = 32 * g
        for q in range(2):
            c = 512 * q
            dwa[:, r: r + 32, c: c + 512] = blk[:, g + 4 * q, None, :]
            dws[:, 2 * g, c: c + 512] = blk[:, g + 4 * q, :]
            dws[:, 2 * g + 1, c: c + 512] = 1.0
    dwa = dwa.transpose(1, 0, 2).reshape(128, T * 1024)
    dws = dws.transpose(1, 0, 2).reshape(8, T * 1024)
    return (np.ascontiguousarray(dwa).astype(BF16),
            np.ascontiguousarray(dws).astype(BF16))


# ---------------------------------------------------------------------------
# numpy emulation (exact fp32 algebra; validates packing + slot maps)
# ---------------------------------------------------------------------------

def emulate_core(inp, core, t_steps=T):
    wb = _weight_blocks(inp)
    ckf = pack_weights_f32(inp)
    x0p = pack_x0(np.asarray(inp["X0"], F32), core, wb["by3"]).astype(F32)
    b3t = pack_b3t(wb["b3"]).astype(F32)
    dwa, dws = pack_dw_rep(np.asarray(inp["dw"], F32), core)
    dwa = dwa.astype(F32).reshape(128, T, 1024).transpose(1, 0, 2)
    dws = dws.astype(F32).reshape(8, T, 1024).transpose(1, 0, 2)

    def sl(x, g, q):
        return x[32 * g: 32 * g + 32, 512 * q: 512 * (q + 1)]

    def bias(col):
        return ckf[:, col: col + 1]

    lacc = np.zeros((128, 128), F32)
    eacc = np.zeros((128, 8), F32)
    ones_row = np.ones((1, BK), F32)

    PST = np.zeros((128, 1024), F32)
    for p in range(4):
        for q in range(2):
            sl(PST, p, q)[:] = wb["WYP"].T @ sl(x0p, p, q)
    XY = PST.copy()
    ph = np.zeros_like(PST)
    for p in range(4):
        for q in range(2):
            sl(ph, p, q)[:] = wb["WY1"].T @ sl(XY, p, q)
    H1 = np.tanh(ph + bias(C_BY1))
    for p in range(4):
        for q in range(2):
            sl(ph, p, q)[:] = wb["WY2"].T @ sl(H1, p, q)
    H2 = np.tanh(ph + bias(C_BY2))
    for p in range(4):
        for q in range(2):
            sl(PST, p, q)[:] += wb["WY3"].T @ sl(H2, p, q)

    for t in range(t_steps):
        XY = PST.copy()
        ph1 = np.zeros_like(PST)
        for p in range(4):
            for q in range(2):
                sl(ph1, _f1(p, q), q)[:] = wb["W1"].T @ sl(XY, p, q)
        H1 = np.tanh(ph1 + bias(C_B1 + t))
        ph2 = np.zeros_like(PST)
        for p in range(4):
            for q in range(2):
                sl(ph2, _f2(p, q), q)[:] = wb["W2"].T @ sl(H1, _f1(p, q), q)
        H2 = np.tanh(ph2 + bias(C_B2))
        pzu = np.zeros_like(PST)
        for p in range(4):
            for q in range(2):
                sl(pzu, _lz(p, q), q)[:] = (wb["W3"].T @ sl(H2, _f2(p, q), q)
                                            + wb["WZY"].T @ sl(XY, p, q))
        ZU = pzu + b3t
        for q in range(2):
            h = ZU[:, 512 * q: 512 * (q + 1)]
            lacc[:, 2 * t + q: 2 * t + q + 1] = np.sum(
                (_ct(t) * h) * h, axis=1, keepdims=True)
        SCA = XY * dwa[t]
        SCB = ZU * dwa[t]
        for p in range(4):
            for q in range(2):
                rhs_sg = dws[t][2 * p: 2 * p + 2,
                                512 * q: 512 * (q + 1)]
                sl(PST, p, q)[:] += (wb["WDXY"].T @ sl(XY, p, q)
                                     + wb["WDZU"].T @ sl(ZU, _lz(p, q), q)
                                     + wb["WDSA"].T @ sl(SCA, p, q)
                                     + wb["WDSB"].T @ sl(SCB, _lz(p, q), q)
                                     + wb["SGR"].T @ rhs_sg)
    XY = PST.copy()
    pe = np.zeros_like(PST)
    for p in range(4):
        for q in range(2):
            sl(pe, p, q)[:] = wb["WE"].T @ sl(XY, p, q)
    eacc[:, 0: 1] = np.sum(pe * pe, axis=1, keepdims=True)
    return lacc, eacc


def reduce_outputs(laccs, eaccs, t_steps=T):
    dh_rows = np.zeros(128, bool)
    e_rows = np.zeros(128, bool)
    for p in range(4):
        dh_rows[32 * p + 24: 32 * p + 32] = True
        e_rows[32 * p: 32 * p + 16] = True
    lc = 0.0
    lb = 0.0
    for lacc, eacc in zip(laccs, eaccs):
        lc += float(np.sum(np.asarray(lacc, np.float64)[dh_rows,
                                                        :2 * t_steps]))
        lb += float(np.sum(np.asarray(eacc, np.float64)[e_rows, 0]))
    return np.array([lb / BATCH, lc / BATCH], F32)


# ---------------------------------------------------------------------------
# device program
# ---------------------------------------------------------------------------

_BUILT = {}


def build(t_steps=T):
    if t_steps in _BUILT:
        return _BUILT[t_steps]
    from contextlib import ExitStack
    import concourse.tile as tile
    from concourse import bacc, mybir

    f32 = mybir.dt.float32
    bf16 = mybir.dt.bfloat16
    AF = mybir.ActivationFunctionType
    OP = mybir.AluOpType

    nc = bacc.Bacc("TRN2", target_bir_lowering=False, debug=False)
    dwa_d = nc.dram_tensor("dwa", [128, T * 1024], bf16,
                           kind="ExternalInput").ap()
    dws_d = nc.dram_tensor("dws", [8, T * 1024], bf16,
                           kind="ExternalInput").ap()
    x0p_d = nc.dram_tensor("x0p", [128, 1024], bf16,
                           kind="ExternalInput").ap()
    b3t_d = nc.dram_tensor("b3t", [128, 1024], bf16, kind="ExternalInput").ap()
    ckb_d = nc.dram_tensor("ckb", [128, K_COLS], bf16,
                           kind="ExternalInput").ap()
    ckf_d = nc.dram_tensor("ckf", [128, C_COLS], f32,
                           kind="ExternalInput").ap()
    lacc_d = nc.dram_tensor("out_lacc", [128, 128], f32,
                            kind="ExternalOutput").ap()
    eacc_d = nc.dram_tensor("out_eacc", [128, 8], f32,
                            kind="ExternalOutput").ap()

    def SL(tens, g, q):
        return tens[32 * g: 32 * g + 32, 512 * q: 512 * (q + 1)]

    with tile.TileContext(nc) as tc, ExitStack() as ctx:
        sb = ctx.enter_context(tc.tile_pool(name="sb", bufs=1))
        dwpool = ctx.enter_context(tc.tile_pool(name="dwp", bufs=3))
        ps = ctx.enter_context(tc.tile_pool(name="ps", bufs=1, space="PSUM"))

        ckb = sb.tile([128, K_COLS], bf16, tag="ckb")
        ckf = sb.tile([128, C_COLS], f32, tag="ckf")
        nc.sync.dma_start(out=ckb[:, :], in_=ckb_d[:, :])
        nc.sync.dma_start(out=ckf[:, :], in_=ckf_d[:, :])
        X0SB = sb.tile([128, 1024], bf16, tag="X0SB")
        nc.sync.dma_start(out=X0SB[:, :], in_=x0p_d[:, :])
        B3T = sb.tile([128, 1024], bf16, tag="B3T")
        nc.sync.dma_start(out=B3T[:, :], in_=b3t_d[:, :])
        XY = sb.tile([128, 1024], bf16, tag="XY")
        H1 = sb.tile([128, 1024], bf16, tag="H1")
        H2 = sb.tile([128, 1024], bf16, tag="H2")
        ZUsb = sb.tile([128, 1024], bf16, tag="ZUsb")
        SCA = sb.tile([128, 1024], bf16, tag="SCA")
        SCB = sb.tile([128, 1024], bf16, tag="SCB")
        SCR = sb.tile([128, 1024], bf16, tag="SCR")
        lacc = sb.tile([128, 128], f32, tag="lacc")
        eacc = sb.tile([128, 8], f32, tag="eacc")
        nc.vector.memset(lacc[:, :], 0.0)
        nc.vector.memset(eacc[:, :], 0.0)

        PST = ps.tile([128, 1024], f32, tag="pst")   # persistent state

        def wkb(off, g):
            return ckb[32 * g: 32 * g + 32, off: off + 32]

        def bias(col):
            return ckf[:, col: col + 1]

        def mm(out_t, og, oq, lhsT, rhs, start, stop, rg):
            nc.tensor.matmul(out=SL(out_t, og, oq), lhsT=lhsT, rhs=rhs,
                             start=start, stop=stop,
                             tile_position=(32 * rg, 32 * og),
                             skip_group_check=True)

        # ---- init: PST = [X0 + by3 rows; += Y0MLP(X0)] ----
        for p in range(4):
            for q in range(2):
                mm(PST, p, q, wkb(K_WYP, p), SL(X0SB, p, q), True, False, p)
        nc.scalar.activation(out=XY[:, :], in_=PST[:, :], func=AF.Copy)
        ph1 = ps.tile([128, 1024], f32, tag="ph1")
        for p in range(4):
            for q in range(2):
                mm(ph1, p, q, wkb(K_WY1, p), SL(XY, p, q), True, True, p)
        nc.scalar.activation(out=H1[:, :], in_=ph1[:, :], func=AF.Tanh,
                             bias=bias(C_BY1))
        ph2 = ps.tile([128, 1024], f32, tag="ph2")
        for p in range(4):
            for q in range(2):
                mm(ph2, p, q, wkb(K_WY2, p), SL(H1, p, q), True, True, p)
        nc.scalar.activation(out=H2[:, :], in_=ph2[:, :], func=AF.Tanh,
                             bias=bias(C_BY2))
        for p in range(4):
            for q in range(2):
                mm(PST, p, q, wkb(K_WY3, p), SL(H2, p, q), False, False, p)

        # ---- steps ----
        for t in range(t_steps):
            dwat = dwpool.tile([128, 1024], bf16, tag="dwa")
            dwst = dwpool.tile([98, 1024], bf16, tag="dws")
            nc.sync.dma_start(out=dwat[:, :],
                              in_=dwa_d[:, 1024 * t: 1024 * (t + 1)])
            for g in range(4):
                nc.sync.dma_start(
                    out=dwst[32 * g: 32 * g + 2, :],
                    in_=dws_d[2 * g: 2 * g + 2, 1024 * t: 1024 * (t + 1)])

            ph1 = ps.tile([128, 1024], f32, tag="ph1")
            ph2 = ps.tile([128, 1024], f32, tag="ph2")
            pzu = ps.tile([128, 1024], f32, tag="pzu")
            last = (t == t_steps - 1)
            H = [(512 * q, 512 * (q + 1)) for q in range(2)]

            nc.scalar.activation(out=XY[:, H[0][0]: H[0][1]],
                                 in_=PST[:, H[0][0]: H[0][1]], func=AF.Copy)
            nc.vector.tensor_copy(out=XY[:, H[1][0]: H[1][1]],
                                  in_=PST[:, H[1][0]: H[1][1]])
            for q in range(2):
                nc.gpsimd.tensor_tensor(out=SCA[:, H[q][0]: H[q][1]],
                                        in0=XY[:, H[q][0]: H[q][1]],
                                        in1=dwat[:, H[q][0]: H[q][1]],
                                        op=OP.mult)
            for q in range(2):
                for p in range(4):
                    mm(pzu, p, q, wkb(K_WZY, p), SL(XY, p, q),
                       True, False, p)
            for q in range(2):
                for p in range(4):
                    mm(ph1, _f1(p, q), q, wkb(K_W1, p), SL(XY, p, q),
                       True, True, p)
            # drift of X/Y from state: fires as soon as XY is ready
            for q in range(2):
                for p in range(4):
                    mm(PST, p, q, wkb(K_WDXY, p), SL(XY, p, q),
                       False, False, p)
            for q in range(2):
                nc.scalar.activation(out=H1[:, H[q][0]: H[q][1]],
                                     in_=ph1[:, H[q][0]: H[q][1]],
                                     func=AF.Tanh, bias=bias(C_B1 + t))
            for q in range(2):
                for p in range(4):
                    mm(ph2, _f2(p, q), q, wkb(K_W2, _f1(p, q)),
                       SL(H1, _f1(p, q), q), True, True, _f1(p, q))
            for q in range(2):
                for p in range(4):
                    mm(PST, p, q, wkb(K_WDSA, p), SL(SCA, p, q),
                       False, False, p)
            for q in range(2):
                nc.scalar.activation(out=H2[:, H[q][0]: H[q][1]],
                                     in_=ph2[:, H[q][0]: H[q][1]],
                                     func=AF.Tanh, bias=bias(C_B2))
            for q in range(2):
                for p in range(4):
                    mm(pzu, p, q, wkb(K_W3, _f2(p, q)),
                       SL(H2, _f2(p, q), q), False, p == 3, _f2(p, q))
            # ZU = pzu + b3; diffusion scale; per-q so the state-update
            # wave of q0 unblocks while q1 is still in flight
            for q in range(2):
                c0, c1 = H[q]
                nc.vector.tensor_tensor(out=ZUsb[:, c0: c1],
                                        in0=pzu[:, c0: c1],
                                        in1=B3T[:, c0: c1], op=OP.add)
                nc.vector.tensor_tensor(out=SCB[:, c0: c1],
                                        in0=ZUsb[:, c0: c1],
                                        in1=dwat[:, c0: c1], op=OP.mult)
            for q in range(2):
                c0, c1 = H[q]
                for p in range(4):
                    mm(PST, p, q, wkb(K_WDZU, p),
                       SL(ZUsb, p, q), False, False, p)
                    mm(PST, p, q, wkb(K_WDSB, p),
                       SL(SCB, p, q), False, False, p)
                    nc.tensor.matmul(
                        out=SL(PST, p, q),
                        lhsT=ckb[32 * p: 32 * p + 2, K_SGR: K_SGR + 32],
                        rhs=dwst[32 * p: 32 * p + 2, c0: c1],
                        start=False, stop=(last and p == 3 and q == 1),
                        tile_position=(32 * p, 32 * p),
                        skip_group_check=True)
            # loss gates nothing downstream: accumulate after the wave
            for q in range(2):
                c0, c1 = H[q]
                nc.vector.scalar_tensor_tensor(
                    out=SCR[:, c0: c1], in0=ZUsb[:, c0: c1],
                    scalar=float(_ct(t)), in1=ZUsb[:, c0: c1],
                    op0=OP.mult, op1=OP.mult,
                    accum_out=lacc[:, 2 * t + q: 2 * t + q + 1])

        # ---- final ----
        nc.scalar.activation(out=XY[:, :], in_=PST[:, :], func=AF.Copy)
        pe = ps.tile([128, 1024], f32, tag="ph1")
        for p in range(4):
            for q in range(2):
                mm(pe, p, q, wkb(K_WE, p), SL(XY, p, q), True, True, p)
        nc.scalar.activation(out=SCR[:, :], in_=pe[:, :], func=AF.Square,
                             accum_out=eacc[:, 0: 1])
        nc.sync.dma_start(out=lacc_d[:, :], in_=lacc[:, :])
        nc.sync.dma_start(out=eacc_d[:, :], in_=eacc[:, :])

    nc.compile()
    _BUILT[t_steps] = nc
    return nc


def make_in_maps(inputs):
    wb = _weight_blocks(inputs)
    ckb = pack_weights_bf16(inputs)
    ckf = pack_weights_f32(inputs)
    b3t = pack_b3t(wb["b3"])
    X0 = np.asarray(inputs["X0"], F32)
    dw = np.asarray(inputs["dw"], F32)
    in_maps = []
    for k in range(NCORES):
        dwa, dws = pack_dw_rep(dw, k)
        in_maps.append({
            "dwa": dwa,
            "dws": dws,
            "x0p": pack_x0(X0, k, wb["by3"]),
            "b3t": b3t,
            "ckb": ckb,
            "ckf": ckf,
        })
    return in_maps


def kernel(**inputs):
    from concourse.bass_utils import run_bass_kernel_spmd

    in_maps = make_in_maps(inputs)
    nc = build(T)
    res = run_bass_kernel_spmd(nc, in_maps, core_ids=list(range(NCORES)))
    laccs = [r["out_lacc"] for r in res.results]
    eaccs = [r["out_eacc"] for r in res.results]
    return reduce_outputs(laccs, eaccs)


if __name__ == "__main__":
    print("module ok")



# revision 4
# speedup vs baseline: 1.2107x; 1.2107x over previous
"""Trainium2 Bass kernel for nn_DeepModel_70703751626759 (deep-BSDE forward sim).

v4: block-diagonal full-array (K=128, M=128) matmuls replace the 32x32
tile-position waves -- one MM per stream type per chain-step (16/step vs
72), same array throughput, far fewer instructions and no concurrency
scheduling.  The two 512-column halves are fully independent pipelines
("chains") with separate SBUF tiles and PSUM banks, emitted c0-then-c1
each step so the scheduler slides them half a step apart.  b3 is folded
into W3 via a tanh-bias ones-row (bias=20 -> tanh=1.0); sigma*dw and
gamma*DT ride the WDSA matmul via host-packed dw rows DMA'd into SCA
rows 24:29.  A PE warmup burst at init gets the HAM clock to 2.4 GHz.

Data-parallel over 8 NeuronCores: 32768 samples -> 4096/core -> 2 chains
x 4 slots x 512 samples.  Slot g of chain c = sample block (g + 4c)*512;
all tiles keep slot g's rows at partition group g (rows 32g:32g+32).
Group content: XY=[X(16);Y(16)], H=[h(20);ones(1);...], ZU=[u(8);Zv(16);
dH(8)].  PST (state) stays fp32 in a persistent PSUM bank per chain.
"""

import sys
import numpy as np

if "/opt/trn_rl_repo" not in sys.path:
    sys.path.insert(0, "/opt/trn_rl_repo")

N = 16
M = 8
T = 50
DT = 0.01
GAMMA = 0.1
SIGMA = 0.2
TAU = 0.5
H = 10
BATCH = 32768
NCORES = 8
CB = BATCH // NCORES      # 4096 samples per core
BK = 512                  # samples per slot
NCH = 2                   # chains per core

F32 = np.float32
try:
    import ml_dtypes
    BF16 = ml_dtypes.bfloat16
except ImportError:          # pragma: no cover
    BF16 = np.float32

ONES_BIAS = F32(20.0)        # tanh(20) = 1.0 to bf16/fp32 precision

# fp32 consts image (ckf) column offsets
C_B1 = 0            # 50 cols: tanh1 bias per t (b1 + t*W1row0); rows 0:20/grp
C_B2 = 50           # tanh2 bias (b2, +20.0 at row 20)
C_BY1 = 51          # Y0-MLP tanh1 bias
C_BY2 = 52          # Y0-MLP tanh2 bias (+20.0 at row 20)
C_COLS = 64


def _ct(t):
    w = 1.0 if (t == 0 or t == T - 1) else 2.0
    return 0.5 * DT * w * TAU * TAU


def _weight_blocks(inp):
    """Per-slot 32x32 (or smaller) weight blocks, v3-proven algebra."""
    A = np.asarray(inp["A"], F32)
    Bm = np.asarray(inp["Bmat"], F32)
    C = np.asarray(inp["Cmat"], F32)
    D = np.asarray(inp["Dmat"], F32)
    ZW1 = np.asarray(inp["Z_W1"], F32)
    ZW2 = np.asarray(inp["Z_W2"], F32)
    ZW3 = np.asarray(inp["Z_W3"], F32)
    PW1 = np.asarray(inp["phi_W1"], F32)
    PW2 = np.asarray(inp["phi_W2"], F32)
    PW3 = np.asarray(inp["phi_W3"], F32)
    YW1 = np.asarray(inp["Y0_W1"], F32)
    YW2 = np.asarray(inp["Y0_W2"], F32)
    YW3 = np.asarray(inp["Y0_W3"], F32)
    I16 = np.eye(16, dtype=F32)

    def blk():
        return np.zeros((32, 32), F32)

    # MLP layer 1: X rows -> [hZ(10) | hphi(10)]
    W1 = blk()
    W1[0:16, 0:10] = ZW1[1:, :]
    W1[0:16, 10:20] = PW1[1:, :]
    # MLP layer 2: H1 rows 0:20 -> ph2 rows 0:20
    W2 = blk()
    W2[0:10, 0:10] = ZW2
    W2[10:20, 10:20] = PW2
    # MLP layer 3 + b3 ones-row: H2 rows 0:21 -> ZU rows [u(0:8);Zv(8:24);dH(24:32)]
    b3u = np.asarray(inp["phi_b3"], F32)
    b3z = np.asarray(inp["Z_b3"], F32)
    W3 = blk()
    W3[10:20, 0:8] = PW3
    W3[0:10, 8:24] = ZW3
    W3[0:10, 24:32] = ZW3 @ D
    W3[10:20, 24:32] = PW3
    W3[20, 0:8] = b3u
    W3[20, 8:24] = b3z
    W3[20, 24:32] = b3u + D.T @ b3z
    # pzu init from XY: Y rows -> dH rows (Y @ Bmat)
    WZY = blk()
    WZY[16:32, 24:32] = Bm
    # state drift from XY
    WDXY = blk()
    WDXY[0:16, 0:16] = DT * A.T
    WDXY[0:16, 16:32] = -DT * I16
    WDXY[16:32, 16:32] = -DT * A
    # diffusion from SCA = dw*X (+ sigma/gamma rows handled at bd level)
    WDSA = blk()
    WDSA[0:16, 0:16] = C.T
    # state drift from ZU
    WDZU = blk()
    WDZU[0:8, 0:16] = DT * Bm.T
    WDZU[8:24, 16:32] = -DT * C
    # diffusion from SCB = dw*ZU
    WDSB = blk()
    WDSB[0:8, 0:16] = D.T
    WDSB[8:24, 16:32] = I16
    # final error: [X;Y] -> rows 0:16 = Y - X
    WE = blk()
    WE[0:16, 0:16] = -I16
    WE[16:32, 0:16] = I16
    # init: X0 rows -> X rows
    WX0 = blk()
    WX0[0:16, 0:16] = I16
    # Y0 MLP (with bY3 via ones-row 20)
    WY1 = blk()
    WY1[0:16, 0:10] = YW1
    WY2 = blk()
    WY2[0:10, 0:10] = YW2
    WY3 = blk()
    WY3[0:10, 16:32] = YW3
    WY3[20, 16:32] = np.asarray(inp["Y0_b3"], F32)
    return dict(W1=W1, W2=W2, W3=W3, WZY=WZY, WDXY=WDXY, WDSA=WDSA,
                WDZU=WDZU, WDSB=WDSB, WE=WE, WX0=WX0, WY1=WY1, WY2=WY2,
                WY3=WY3)


BD_NAMES = ["W1", "W2", "W3", "WZY", "WDXY", "WDSA", "WDZU", "WDSB",
            "WE", "WX0", "WY1", "WY2", "WY3"]


def pack_weights_bd(inp):
    """Block-diagonal [128,128] bf16 images, one per stream type."""
    wb = _weight_blocks(inp)
    out = {}
    for name in BD_NAMES:
        img = np.zeros((128, 128), F32)
        for g in range(4):
            img[32 * g: 32 * g + 32, 32 * g: 32 * g + 32] = wb[name]
        out[name] = img
    # WDSA extra: SCA group-0 rows 24:29 = [dw_s0..dw_s3, 1] -> sigma/gammaDT
    bd = out["WDSA"]
    for p in range(4):
        bd[24 + p, 32 * p: 32 * p + 16] = SIGMA
        bd[28, 32 * p: 32 * p + 16] = GAMMA * DT
    return {k: v.astype(BF16) for k, v in out.items()}


def pack_weights_f32(inp):
    """Bias image: per-partition fp32 columns."""
    Zb1 = np.asarray(inp["Z_b1"], F32)
    Pb1 = np.asarray(inp["phi_b1"], F32)
    Zb2 = np.asarray(inp["Z_b2"], F32)
    Pb2 = np.asarray(inp["phi_b2"], F32)
    Yb1 = np.asarray(inp["Y0_b1"], F32)
    Yb2 = np.asarray(inp["Y0_b2"], F32)
    ZW1 = np.asarray(inp["Z_W1"], F32)
    PW1 = np.asarray(inp["phi_W1"], F32)
    img = np.zeros((128, C_COLS), F32)
    b2 = np.concatenate([Zb2, Pb2])
    for g in range(4):
        r = 32 * g
        for t in range(T):
            tv = F32(t * DT)
            img[r: r + 10, C_B1 + t] = Zb1 + tv * ZW1[0, :]
            img[r + 10: r + 20, C_B1 + t] = Pb1 + tv * PW1[0, :]
        img[r: r + 20, C_B2] = b2
        img[r + 20, C_B2] = ONES_BIAS
        img[r: r + 10, C_BY1] = Yb1
        img[r: r + 10, C_BY2] = Yb2
        img[r + 20, C_BY2] = ONES_BIAS
    return img


def pack_x0(X0, core):
    """Per-chain X0 tiles: [2][128, 512] bf16; slot g rows 0:16 = X0 block.T"""
    base = core * CB
    out = np.zeros((NCH, 128, BK), F32)
    for c in range(NCH):
        for g in range(4):
            j = g + 4 * c
            out[c, 32 * g: 32 * g + 16, :] = \
                X0[base + BK * j: base + BK * (j + 1), :].T
    return np.ascontiguousarray(out.astype(BF16))


def pack_dwat(dw, core):
    """dwat [128, T*1024]: cols t*1024 + 512c = step t, chain c.

    rows 32g:32g+24: dw of slot (g,c) replicated (for SCA/SCB elementwise)
    rows 24:29 (group 0 only): [dw_s0, dw_s1, dw_s2, dw_s3, 1]  (sigma/gamma)
    """
    base = core * CB
    d = np.asarray(dw, F32)[:, base: base + CB, 0]      # [T, CB]
    blk = d.reshape(T, 8, BK)                           # slot j = g + 4c
    out = np.empty((128, T, NCH, BK), F32)
    for c in range(NCH):
        for g in range(4):
            r = 32 * g
            out[r: r + 32, :, c, :] = blk[None, :, g + 4 * c, :]
        for j in range(4):
            out[24 + j, :, c, :] = blk[:, j + 4 * c, :]
        out[28, :, c, :] = 1.0
        out[29: 32, :, c, :] = 0.0
    out = out.reshape(128, T * NCH * BK)
    return np.ascontiguousarray(out.astype(BF16))


# ---------------------------------------------------------------------------
# numpy emulation (validates packing + algebra; bf16 casts where HW casts)
# ---------------------------------------------------------------------------

def _b(x):
    return x.astype(BF16).astype(F32)


def emulate_core(inp, core, t_steps=T):
    bd = {k: v.astype(F32) for k, v in pack_weights_bd(inp).items()}
    ckf = pack_weights_f32(inp)
    x0p = pack_x0(np.asarray(inp["X0"], F32), core).astype(F32)
    dwat = pack_dwat(np.asarray(inp["dw"], F32), core).astype(F32)
    dwat = dwat.reshape(128, T, NCH, BK)

    def bias(col):
        return ckf[:, col: col + 1]

    lacc = np.zeros((128, 128), F32)
    eacc = np.zeros((128, 8), F32)

    for c in range(NCH):
        # init
        PST = bd["WX0"].T @ x0p[c]
        ph1 = bd["WY1"].T @ x0p[c]
        H1 = _b(np.tanh(ph1 + bias(C_BY1)))
        ph2 = bd["WY2"].T @ H1
        H2 = _b(np.tanh(ph2 + bias(C_BY2)))
        PST = PST + bd["WY3"].T @ H2

        for t in range(t_steps):
            dwt = dwat[:, t, c, :]
            XY = _b(PST)
            SCA = _b(XY * dwt)
            SCA[24:29, :] = dwt[24:29, :]            # sigma/gamma dw rows (DMA)
            ph1 = bd["W1"].T @ XY
            pzu = bd["WZY"].T @ XY
            PSTn = PST + bd["WDXY"].T @ XY + bd["WDSA"].T @ SCA
            H1 = _b(np.tanh(ph1 + bias(C_B1 + t)))
            ph2 = bd["W2"].T @ H1
            H2 = _b(np.tanh(ph2 + bias(C_B2)))
            pzu = pzu + bd["W3"].T @ H2
            ZU = _b(pzu)
            SCB = _b(ZU * dwt)
            PSTn = PSTn + bd["WDZU"].T @ ZU + bd["WDSB"].T @ SCB
            lacc[:, 2 * t + c] += np.sum(
                (F32(_ct(t)) * ZU) * ZU, axis=1)
            PST = PSTn

        XY = _b(PST)
        pe = bd["WE"].T @ XY
        eacc[:, c] = np.sum(pe * pe, axis=1)
    return lacc, eacc


def reduce_outputs(laccs, eaccs, t_steps=T):
    dh_rows = np.zeros(128, bool)
    e_rows = np.zeros(128, bool)
    for g in range(4):
        dh_rows[32 * g + 24: 32 * g + 32] = True
        e_rows[32 * g: 32 * g + 16] = True
    lc = 0.0
    lb = 0.0
    for lacc, eacc in zip(laccs, eaccs):
        lc += float(np.sum(np.asarray(lacc, np.float64)[dh_rows,
                                                        :2 * t_steps]))
        lb += float(np.sum(np.asarray(eacc, np.float64)[e_rows, 0:NCH]))
    return np.array([lb / BATCH, lc / BATCH], F32)


# ---------------------------------------------------------------------------
# device program
# ---------------------------------------------------------------------------

_BUILT = {}


def build(t_steps=T):
    if t_steps in _BUILT:
        return _BUILT[t_steps]
    from contextlib import ExitStack
    import concourse.tile as tile
    from concourse import bacc, mybir

    f32 = mybir.dt.float32
    bf16 = mybir.dt.bfloat16
    AF = mybir.ActivationFunctionType
    OP = mybir.AluOpType

    nc = bacc.Bacc("TRN2", target_bir_lowering=False, debug=False)
    dwat_d = nc.dram_tensor("dwat", [128, T * NCH * BK], bf16,
                            kind="ExternalInput").ap()
    x0p_d = nc.dram_tensor("x0p", [NCH * 128, BK], bf16,
                           kind="ExternalInput").ap()
    ckb_d = nc.dram_tensor("ckb", [len(BD_NAMES) * 128, 128], bf16,
                           kind="ExternalInput").ap()
    ckf_d = nc.dram_tensor("ckf", [128, C_COLS], f32,
                           kind="ExternalInput").ap()
    lacc_d = nc.dram_tensor("out_lacc", [128, 128], f32,
                            kind="ExternalOutput").ap()
    eacc_d = nc.dram_tensor("out_eacc", [128, 8], f32,
                            kind="ExternalOutput").ap()

    with tile.TileContext(nc) as tc, ExitStack() as ctx:
        sb = ctx.enter_context(tc.tile_pool(name="sb", bufs=1))
        dwpool = ctx.enter_context(tc.tile_pool(name="dwp", bufs=3))
        ps = ctx.enter_context(tc.tile_pool(name="ps", bufs=1, space="PSUM"))

        W = {}
        for i, name in enumerate(BD_NAMES):
            W[name] = sb.tile([128, 128], bf16, tag=f"w_{name}", name=f"w_{name}")
            nc.sync.dma_start(out=W[name][:, :],
                              in_=ckb_d[128 * i: 128 * (i + 1), :])
        ckf = sb.tile([128, C_COLS], f32, tag="ckf")
        nc.sync.dma_start(out=ckf[:, :], in_=ckf_d[:, :])

        def bias(col):
            return ckf[:, col: col + 1]

        X0SB, XY, H1, H2, ZUsb, SCA, SCB, SCR = ([None, None]
                                                 for _ in range(8))
        PST, ph1, ph2, pzu = [None, None], [None, None], [None, None], \
            [None, None]
        for c in range(NCH):
            X0SB[c] = sb.tile([128, BK], bf16, tag=f"x0_{c}", name=f"x0_{c}")
            nc.sync.dma_start(out=X0SB[c][:, :],
                              in_=x0p_d[128 * c: 128 * (c + 1), :])
            XY[c] = sb.tile([128, BK], bf16, tag=f"xy_{c}", name=f"xy_{c}")
            H1[c] = sb.tile([128, BK], bf16, tag=f"h1_{c}", name=f"h1_{c}")
            H2[c] = sb.tile([128, BK], bf16, tag=f"h2_{c}", name=f"h2_{c}")
            ZUsb[c] = sb.tile([128, BK], bf16, tag=f"zu_{c}", name=f"zu_{c}")
            SCA[c] = sb.tile([128, BK], bf16, tag=f"sca_{c}", name=f"sca_{c}")
            SCB[c] = sb.tile([128, BK], bf16, tag=f"scb_{c}", name=f"scb_{c}")
            SCR[c] = sb.tile([128, BK], bf16, tag=f"scr_{c}", name=f"scr_{c}")
            PST[c] = ps.tile([128, BK], f32, tag=f"pst_{c}", name=f"pst_{c}")
            ph1[c] = ps.tile([128, BK], f32, tag=f"ph1_{c}", name=f"ph1_{c}")
            ph2[c] = ps.tile([128, BK], f32, tag=f"ph2_{c}", name=f"ph2_{c}")
            pzu[c] = ps.tile([128, BK], f32, tag=f"pzu_{c}", name=f"pzu_{c}")
        lacc = sb.tile([128, 128], f32, tag="lacc")
        eacc = sb.tile([128, 8], f32, tag="eacc")
        nc.vector.memset(lacc[:, :], 0.0)
        nc.vector.memset(eacc[:, :], 0.0)

        def mm(out_t, w, rhs, start, stop):
            nc.tensor.matmul(out=out_t[:, :], lhsT=W[w][:, :], rhs=rhs[:, :],
                             start=start, stop=stop, skip_group_check=True)

        # ---- PE warmup burst: ~18 back-to-back MMs (~3.8us cold) so the
        # HAM clock reaches 8/8 before the real pipeline starts ----
        for c in range(NCH):
            for k in range(9):
                mm(ph1[c], "WX0", X0SB[c], True, True)

        # ---- init: PST = [X0; Y0MLP(X0)] per chain ----
        for c in range(NCH):
            mm(PST[c], "WX0", X0SB[c], True, False)
            mm(ph1[c], "WY1", X0SB[c], True, True)
            nc.scalar.activation(out=H1[c][:, :], in_=ph1[c][:, :],
                                 func=AF.Tanh, bias=bias(C_BY1))
            mm(ph2[c], "WY2", H1[c], True, True)
            nc.scalar.activation(out=H2[c][:, :], in_=ph2[c][:, :],
                                 func=AF.Tanh, bias=bias(C_BY2))
            mm(PST[c], "WY3", H2[c], False, False)

        # ---- steps ----
        for t in range(t_steps):
            dwt = dwpool.tile([128, NCH * BK], bf16, tag="dwa")
            nc.sync.dma_start(
                out=dwt[:, :],
                in_=dwat_d[:, NCH * BK * t: NCH * BK * (t + 1)])
            for c in range(NCH):
                last = (t == t_steps - 1)
                dws = dwt[:, BK * c: BK * (c + 1)]
                # state copy PSUM -> SBUF bf16 (ACT for c0, DVE for c1)
                if c == 0:
                    nc.scalar.activation(out=XY[c][:, :], in_=PST[c][:, :],
                                         func=AF.Copy)
                else:
                    nc.vector.tensor_copy(out=XY[c][:, :], in_=PST[c][:, :])
                # SCA = XY * dw  (gpsimd), then sigma/gamma rows via DMA
                nc.gpsimd.tensor_tensor(out=SCA[c][:, :], in0=XY[c][:, :],
                                        in1=dws, op=OP.mult)
                nc.sync.dma_start(out=SCA[c][24:29, :], in_=dws[24:29, :])
                # streams from XY
                mm(ph1[c], "W1", XY[c], True, True)
                mm(pzu[c], "WZY", XY[c], True, False)
                mm(PST[c], "WDXY", XY[c], False, False)
                mm(PST[c], "WDSA", SCA[c], False, False)
                # MLP
                nc.scalar.activation(out=H1[c][:, :], in_=ph1[c][:, :],
                                     func=AF.Tanh, bias=bias(C_B1 + t))
                mm(ph2[c], "W2", H1[c], True, True)
                nc.scalar.activation(out=H2[c][:, :], in_=ph2[c][:, :],
                                     func=AF.Tanh, bias=bias(C_B2))
                mm(pzu[c], "W3", H2[c], False, True)
                # ZU out of PSUM; SCB = ZU * dw
                nc.vector.tensor_copy(out=ZUsb[c][:, :], in_=pzu[c][:, :])
                if c == 0:
                    nc.gpsimd.tensor_tensor(out=SCB[c][:, :],
                                            in0=ZUsb[c][:, :],
                                            in1=dws, op=OP.mult)
                else:
                    nc.vector.tensor_tensor(out=SCB[c][:, :],
                                            in0=ZUsb[c][:, :],
                                            in1=dws, op=OP.mult)
                mm(PST[c], "WDZU", ZUsb[c], False, False)
                mm(PST[c], "WDSB", SCB[c], False, last)
                # loss accumulate (off critical path)
                nc.vector.scalar_tensor_tensor(
                    out=SCR[c][:, :], in0=ZUsb[c][:, :],
                    scalar=float(_ct(t)), in1=ZUsb[c][:, :],
                    op0=OP.mult, op1=OP.mult,
                    accum_out=lacc[:, 2 * t + c: 2 * t + c + 1])

        # ---- final ----
        for c in range(NCH):
            nc.scalar.activation(out=XY[c][:, :], in_=PST[c][:, :],
                                 func=AF.Copy)
            mm(ph1[c], "WE", XY[c], True, True)
            nc.scalar.activation(out=SCR[c][:, :], in_=ph1[c][:, :],
                                 func=AF.Square,
                                 accum_out=eacc[:, c: c + 1])
        nc.sync.dma_start(out=lacc_d[:, :], in_=lacc[:, :])
        nc.sync.dma_start(out=eacc_d[:, :], in_=eacc[:, :])

    nc.compile()
    _BUILT[t_steps] = nc
    return nc


def make_in_maps(inputs):
    bd = pack_weights_bd(inputs)
    ckb = np.concatenate([bd[name] for name in BD_NAMES], axis=0)
    ckf = pack_weights_f32(inputs)
    X0 = np.asarray(inputs["X0"], F32)
    dw = np.asarray(inputs["dw"], F32)
    in_maps = []
    for k in range(NCORES):
        x0p = pack_x0(X0, k)
        in_maps.append({
            "dwat": pack_dwat(dw, k),
            "x0p": np.ascontiguousarray(x0p.reshape(NCH * 128, BK)),
            "ckb": np.ascontiguousarray(ckb),
            "ckf": ckf,
        })
    return in_maps


def kernel(**inputs):
    from concourse.bass_utils import run_bass_kernel_spmd

    in_maps = make_in_maps(inputs)
    nc = build(T)
    res = run_bass_kernel_spmd(nc, in_maps, core_ids=list(range(NCORES)))
    laccs = [r["out_lacc"] for r in res.results]
    eaccs = [r["out_eacc"] for r in res.results]
    return reduce_outputs(laccs, eaccs)


if __name__ == "__main__":
    print("module ok")


# revision 5
# speedup vs baseline: 1.8108x; 1.4956x over previous
"""Trainium2 Bass kernel for nn_DeepModel_70703751626759 (deep-BSDE forward sim).

v4: block-diagonal full-array (K=128, M=128) matmuls replace the 32x32
tile-position waves -- one MM per stream type per chain-step (16/step vs
72), same array throughput, far fewer instructions and no concurrency
scheduling.  The two 512-column halves are fully independent pipelines
("chains") with separate SBUF tiles and PSUM banks, emitted c0-then-c1
each step so the scheduler slides them half a step apart.  b3 is folded
into W3 via a tanh-bias ones-row (bias=20 -> tanh=1.0); sigma*dw and
gamma*DT ride the WDSA matmul via host-packed dw rows DMA'd into SCA
rows 24:29.  A PE warmup burst at init gets the HAM clock to 2.4 GHz.

Data-parallel over 8 NeuronCores: 32768 samples -> 4096/core -> 2 chains
x 4 slots x 512 samples.  Slot g of chain c = sample block (g + 4c)*512;
all tiles keep slot g's rows at partition group g (rows 32g:32g+32).
Group content: XY=[X(16);Y(16)], H=[h(20);ones(1);...], ZU=[u(8);Zv(16);
dH(8)].  PST (state) stays fp32 in a persistent PSUM bank per chain.
"""

import sys
import numpy as np

if "/opt/trn_rl_repo" not in sys.path:
    sys.path.insert(0, "/opt/trn_rl_repo")

N = 16
M = 8
T = 50
DT = 0.01
GAMMA = 0.1
SIGMA = 0.2
TAU = 0.5
H = 10
BATCH = 32768
NCORES = 8
CB = BATCH // NCORES      # 4096 samples per core
BK = 512                  # samples per slot
NCH = 2                   # chains per core

F32 = np.float32
try:
    import ml_dtypes
    BF16 = ml_dtypes.bfloat16
except ImportError:          # pragma: no cover
    BF16 = np.float32

ONES_BIAS = F32(20.0)        # tanh(20) = 1.0 to bf16/fp32 precision

# fp32 consts image (ckf) column offsets
C_B1 = 0            # 50 cols: tanh1 bias per t (b1 + t*W1row0); rows 0:20/grp
C_B2 = 50           # tanh2 bias (b2, +20.0 at row 20)
C_BY1 = 51          # Y0-MLP tanh1 bias
C_BY2 = 52          # Y0-MLP tanh2 bias (+20.0 at row 20)
C_COLS = 64


def _ct(t):
    w = 1.0 if (t == 0 or t == T - 1) else 2.0
    return 0.5 * DT * w * TAU * TAU


def _weight_blocks(inp):
    """Per-slot 32x32 (or smaller) weight blocks, v3-proven algebra."""
    A = np.asarray(inp["A"], F32)
    Bm = np.asarray(inp["Bmat"], F32)
    C = np.asarray(inp["Cmat"], F32)
    D = np.asarray(inp["Dmat"], F32)
    ZW1 = np.asarray(inp["Z_W1"], F32)
    ZW2 = np.asarray(inp["Z_W2"], F32)
    ZW3 = np.asarray(inp["Z_W3"], F32)
    PW1 = np.asarray(inp["phi_W1"], F32)
    PW2 = np.asarray(inp["phi_W2"], F32)
    PW3 = np.asarray(inp["phi_W3"], F32)
    YW1 = np.asarray(inp["Y0_W1"], F32)
    YW2 = np.asarray(inp["Y0_W2"], F32)
    YW3 = np.asarray(inp["Y0_W3"], F32)
    I16 = np.eye(16, dtype=F32)

    def blk():
        return np.zeros((32, 32), F32)

    # MLP layer 1: X rows -> [hZ(10) | hphi(10)]
    W1 = blk()
    W1[0:16, 0:10] = ZW1[1:, :]
    W1[0:16, 10:20] = PW1[1:, :]
    # MLP layer 2: H1 rows 0:20 -> ph2 rows 0:20
    W2 = blk()
    W2[0:10, 0:10] = ZW2
    W2[10:20, 10:20] = PW2
    # MLP layer 3 + b3 ones-row: H2 rows 0:21 -> ZU rows [u(0:8);Zv(8:24);dH(24:32)]
    b3u = np.asarray(inp["phi_b3"], F32)
    b3z = np.asarray(inp["Z_b3"], F32)
    W3 = blk()
    W3[10:20, 0:8] = PW3
    W3[0:10, 8:24] = ZW3
    W3[0:10, 24:32] = ZW3 @ D
    W3[10:20, 24:32] = PW3
    W3[20, 0:8] = b3u
    W3[20, 8:24] = b3z
    W3[20, 24:32] = b3u + D.T @ b3z
    # pzu init from XY: Y rows -> dH rows (Y @ Bmat)
    WZY = blk()
    WZY[16:32, 24:32] = Bm
    # state drift from XY
    WDXY = blk()
    WDXY[0:16, 0:16] = DT * A.T
    WDXY[0:16, 16:32] = -DT * I16
    WDXY[16:32, 16:32] = -DT * A
    # diffusion from SCA = dw*X (+ sigma/gamma rows handled at bd level)
    WDSA = blk()
    WDSA[0:16, 0:16] = C.T
    # state drift from ZU
    WDZU = blk()
    WDZU[0:8, 0:16] = DT * Bm.T
    WDZU[8:24, 16:32] = -DT * C
    # diffusion from SCB = dw*ZU
    WDSB = blk()
    WDSB[0:8, 0:16] = D.T
    WDSB[8:24, 16:32] = I16
    # final error: [X;Y] -> rows 0:16 = Y - X
    WE = blk()
    WE[0:16, 0:16] = -I16
    WE[16:32, 0:16] = I16
    # init: X0 rows -> X rows
    WX0 = blk()
    WX0[0:16, 0:16] = I16
    # Y0 MLP (with bY3 via ones-row 20)
    WY1 = blk()
    WY1[0:16, 0:10] = YW1
    WY2 = blk()
    WY2[0:10, 0:10] = YW2
    WY3 = blk()
    WY3[0:10, 16:32] = YW3
    WY3[20, 16:32] = np.asarray(inp["Y0_b3"], F32)
    return dict(W1=W1, W2=W2, W3=W3, WZY=WZY, WDXY=WDXY, WDSA=WDSA,
                WDZU=WDZU, WDSB=WDSB, WE=WE, WX0=WX0, WY1=WY1, WY2=WY2,
                WY3=WY3)


BD_NAMES = ["W1", "W2", "W3", "WZY", "WDXY", "WDSA", "WDZU", "WDSB",
            "WE", "WX0", "WY1", "WY2", "WY3"]


def pack_weights_bd(inp):
    """Block-diagonal [128,128] bf16 images, one per stream type."""
    wb = _weight_blocks(inp)
    out = {}
    for name in BD_NAMES:
        img = np.zeros((128, 128), F32)
        for g in range(4):
            img[32 * g: 32 * g + 32, 32 * g: 32 * g + 32] = wb[name]
        out[name] = img
    # WDSA extra: SCA group-0 rows 24:29 = [dw_s0..dw_s3, 1] -> sigma/gammaDT
    bd = out["WDSA"]
    for p in range(4):
        bd[24 + p, 32 * p: 32 * p + 16] = SIGMA
        bd[28, 32 * p: 32 * p + 16] = GAMMA * DT
    return {k: v.astype(BF16) for k, v in out.items()}


def pack_weights_f32(inp):
    """Bias image: per-partition fp32 columns."""
    Zb1 = np.asarray(inp["Z_b1"], F32)
    Pb1 = np.asarray(inp["phi_b1"], F32)
    Zb2 = np.asarray(inp["Z_b2"], F32)
    Pb2 = np.asarray(inp["phi_b2"], F32)
    Yb1 = np.asarray(inp["Y0_b1"], F32)
    Yb2 = np.asarray(inp["Y0_b2"], F32)
    ZW1 = np.asarray(inp["Z_W1"], F32)
    PW1 = np.asarray(inp["phi_W1"], F32)
    img = np.zeros((128, C_COLS), F32)
    b2 = np.concatenate([Zb2, Pb2])
    for g in range(4):
        r = 32 * g
        for t in range(T):
            tv = F32(t * DT)
            img[r: r + 10, C_B1 + t] = Zb1 + tv * ZW1[0, :]
            img[r + 10: r + 20, C_B1 + t] = Pb1 + tv * PW1[0, :]
        img[r: r + 20, C_B2] = b2
        img[r + 20, C_B2] = ONES_BIAS
        img[r: r + 10, C_BY1] = Yb1
        img[r: r + 10, C_BY2] = Yb2
        img[r + 20, C_BY2] = ONES_BIAS
    return img


def pack_x0(X0, core):
    """Per-chain X0 tiles: [2][128, 512] bf16; slot g rows 0:16 = X0 block.T"""
    base = core * CB
    out = np.zeros((NCH, 128, BK), F32)
    for c in range(NCH):
        for g in range(4):
            j = g + 4 * c
            out[c, 32 * g: 32 * g + 16, :] = \
                X0[base + BK * j: base + BK * (j + 1), :].T
    return np.ascontiguousarray(out.astype(BF16))


def pack_dwat(dw, core):
    """dwat [128, T*1024]: cols t*1024 + 512c = step t, chain c.

    rows 32g:32g+24: dw of slot (g,c) replicated (for SCA/SCB elementwise)
    rows 24:29 (group 0 only): [dw_s0, dw_s1, dw_s2, dw_s3, 1]  (sigma/gamma)
    """
    base = core * CB
    d = np.asarray(dw, F32)[:, base: base + CB, 0]      # [T, CB]
    blk = d.reshape(T, 8, BK)                           # slot j = g + 4c
    out = np.empty((128, T, NCH, BK), F32)
    for c in range(NCH):
        for g in range(4):
            r = 32 * g
            out[r: r + 32, :, c, :] = blk[None, :, g + 4 * c, :]
        for j in range(4):
            out[24 + j, :, c, :] = blk[:, j + 4 * c, :]
        out[28, :, c, :] = 1.0
        out[29: 32, :, c, :] = 0.0
    out = out.reshape(128, T * NCH * BK)
    return np.ascontiguousarray(out.astype(BF16))


# ---------------------------------------------------------------------------
# numpy emulation (validates packing + algebra; bf16 casts where HW casts)
# ---------------------------------------------------------------------------

def _b(x):
    return x.astype(BF16).astype(F32)


def emulate_core(inp, core, t_steps=T):
    bd = {k: v.astype(F32) for k, v in pack_weights_bd(inp).items()}
    ckf = pack_weights_f32(inp)
    x0p = pack_x0(np.asarray(inp["X0"], F32), core).astype(F32)
    dwat = pack_dwat(np.asarray(inp["dw"], F32), core).astype(F32)
    dwat = dwat.reshape(128, T, NCH, BK)

    def bias(col):
        return ckf[:, col: col + 1]

    lacc = np.zeros((128, 128), F32)
    eacc = np.zeros((128, 8), F32)

    for c in range(NCH):
        # init
        PST = bd["WX0"].T @ x0p[c]
        ph1 = bd["WY1"].T @ x0p[c]
        H1 = _b(np.tanh(ph1 + bias(C_BY1)))
        ph2 = bd["WY2"].T @ H1
        H2 = _b(np.tanh(ph2 + bias(C_BY2)))
        PST = PST + bd["WY3"].T @ H2

        for t in range(t_steps):
            dwt = dwat[:, t, c, :]
            XY = _b(PST)
            SCA = _b(XY * dwt)
            SCA[24:29, :] = dwt[24:29, :]            # sigma/gamma dw rows (DMA)
            ph1 = bd["W1"].T @ XY
            pzu = bd["WZY"].T @ XY
            PSTn = PST + bd["WDXY"].T @ XY + bd["WDSA"].T @ SCA
            H1 = _b(np.tanh(ph1 + bias(C_B1 + t)))
            ph2 = bd["W2"].T @ H1
            H2 = _b(np.tanh(ph2 + bias(C_B2)))
            pzu = pzu + bd["W3"].T @ H2
            ZU = _b(pzu)
            SCB = _b(ZU * dwt)
            PSTn = PSTn + bd["WDZU"].T @ ZU + bd["WDSB"].T @ SCB
            lacc[:, 2 * t + c] += np.sum(
                (F32(_ct(t)) * ZU) * ZU, axis=1)
            PST = PSTn

        XY = _b(PST)
        pe = bd["WE"].T @ XY
        eacc[:, c] = np.sum(pe * pe, axis=1)
    return lacc, eacc


def reduce_outputs(laccs, eaccs, t_steps=T):
    dh_rows = np.zeros(128, bool)
    e_rows = np.zeros(128, bool)
    for g in range(4):
        dh_rows[32 * g + 24: 32 * g + 32] = True
        e_rows[32 * g: 32 * g + 16] = True
    lc = 0.0
    lb = 0.0
    for lacc, eacc in zip(laccs, eaccs):
        lc += float(np.sum(np.asarray(lacc, np.float64)[dh_rows,
                                                        :2 * t_steps]))
        lb += float(np.sum(np.asarray(eacc, np.float64)[e_rows, 0:NCH]))
    return np.array([lb / BATCH, lc / BATCH], F32)


# ---------------------------------------------------------------------------
# device program
# ---------------------------------------------------------------------------

_BUILT = {}


def build(t_steps=T):
    if t_steps in _BUILT:
        return _BUILT[t_steps]
    from contextlib import ExitStack
    import concourse.tile as tile
    from concourse import bacc, mybir

    f32 = mybir.dt.float32
    bf16 = mybir.dt.bfloat16
    AF = mybir.ActivationFunctionType
    OP = mybir.AluOpType

    nc = bacc.Bacc("TRN2", target_bir_lowering=False, debug=False)
    dwat_d = nc.dram_tensor("dwat", [128, T * NCH * BK], bf16,
                            kind="ExternalInput").ap()
    x0p_d = nc.dram_tensor("x0p", [NCH * 128, BK], bf16,
                           kind="ExternalInput").ap()
    ckb_d = nc.dram_tensor("ckb", [len(BD_NAMES) * 128, 128], bf16,
                           kind="ExternalInput").ap()
    ckf_d = nc.dram_tensor("ckf", [128, C_COLS], f32,
                           kind="ExternalInput").ap()
    lacc_d = nc.dram_tensor("out_lacc", [128, 128], f32,
                            kind="ExternalOutput").ap()
    eacc_d = nc.dram_tensor("out_eacc", [128, 8], f32,
                            kind="ExternalOutput").ap()

    with tile.TileContext(nc) as tc, ExitStack() as ctx:
        sb = ctx.enter_context(tc.tile_pool(name="sb", bufs=1))
        dwpool = ctx.enter_context(tc.tile_pool(name="dwp", bufs=3))
        ps = ctx.enter_context(tc.tile_pool(name="ps", bufs=1, space="PSUM"))

        W = {}
        for i, name in enumerate(BD_NAMES):
            W[name] = sb.tile([128, 128], bf16, tag=f"w_{name}", name=f"w_{name}")
            nc.sync.dma_start(out=W[name][:, :],
                              in_=ckb_d[128 * i: 128 * (i + 1), :])
        ckf = sb.tile([128, C_COLS], f32, tag="ckf")
        nc.sync.dma_start(out=ckf[:, :], in_=ckf_d[:, :])

        def bias(col):
            return ckf[:, col: col + 1]

        X0SB, XY, H1, H2, ZUsb, SCA, SCB, SCR = ([None, None]
                                                 for _ in range(8))
        PST, ph1, ph2, pzu = [None, None], [None, None], [None, None], \
            [None, None]
        for c in range(NCH):
            X0SB[c] = sb.tile([128, BK], bf16, tag=f"x0_{c}", name=f"x0_{c}")
            nc.sync.dma_start(out=X0SB[c][:, :],
                              in_=x0p_d[128 * c: 128 * (c + 1), :])
            XY[c] = sb.tile([128, BK], bf16, tag=f"xy_{c}", name=f"xy_{c}")
            H1[c] = sb.tile([128, BK], bf16, tag=f"h1_{c}", name=f"h1_{c}")
            H2[c] = sb.tile([128, BK], bf16, tag=f"h2_{c}", name=f"h2_{c}")
            ZUsb[c] = sb.tile([128, BK], bf16, tag=f"zu_{c}", name=f"zu_{c}")
            SCA[c] = sb.tile([128, BK], bf16, tag=f"sca_{c}", name=f"sca_{c}")
            SCB[c] = sb.tile([128, BK], bf16, tag=f"scb_{c}", name=f"scb_{c}")
            SCR[c] = sb.tile([128, BK], bf16, tag=f"scr_{c}", name=f"scr_{c}")
            PST[c] = ps.tile([128, BK], f32, tag=f"pst_{c}", name=f"pst_{c}")
            ph1[c] = ps.tile([128, BK], f32, tag=f"ph1_{c}", name=f"ph1_{c}")
            ph2[c] = ps.tile([128, BK], f32, tag=f"ph2_{c}", name=f"ph2_{c}")
            pzu[c] = ps.tile([128, BK], f32, tag=f"pzu_{c}", name=f"pzu_{c}")
        lacc = sb.tile([128, 128], f32, tag="lacc")
        eacc = sb.tile([128, 8], f32, tag="eacc")
        nc.vector.memset(lacc[:, :], 0.0)
        nc.vector.memset(eacc[:, :], 0.0)

        def mm(out_t, w, rhs, start, stop):
            nc.tensor.matmul(out=out_t[:, :], lhsT=W[w][:, :], rhs=rhs[:, :],
                             start=start, stop=stop, skip_group_check=True)

        # ---- PE warmup burst: ~18 back-to-back MMs (~3.8us cold) so the
        # HAM clock reaches 8/8 before the real pipeline starts ----
        for c in range(NCH):
            for k in range(9):
                mm(ph1[c], "WX0", X0SB[c], True, True)

        # ---- init: PST = [X0; Y0MLP(X0)] per chain ----
        for c in range(NCH):
            mm(PST[c], "WX0", X0SB[c], True, False)
            mm(ph1[c], "WY1", X0SB[c], True, True)
            nc.scalar.activation(out=H1[c][:, :], in_=ph1[c][:, :],
                                 func=AF.Tanh, bias=bias(C_BY1))
            mm(ph2[c], "WY2", H1[c], True, True)
            nc.scalar.activation(out=H2[c][:, :], in_=ph2[c][:, :],
                                 func=AF.Tanh, bias=bias(C_BY2))
            mm(PST[c], "WY3", H2[c], False, False)

        # ---- steps: software-pipelined, chains offset half a period ----
        dwtiles = {}

        def dw_slice(t, c):
            return dwtiles[t][:, BK * c: BK * (c + 1)]

        def emit_dma(t):
            dwt = dwpool.tile([128, NCH * BK], bf16, tag="dwa", name="dwa")
            nc.sync.dma_start(
                out=dwt[:, :],
                in_=dwat_d[:, NCH * BK * t: NCH * BK * (t + 1)])
            dwtiles[t] = dwt
            if t - 2 in dwtiles:
                del dwtiles[t - 2]

        def emit_h1(c, t):
            """state copy -> MLP front: ends with W3's input H2 ready."""
            dws = dw_slice(t, c)
            if c == 0:
                nc.scalar.activation(out=XY[c][:, :], in_=PST[c][:, :],
                                     func=AF.Copy)
            else:
                nc.vector.tensor_copy(out=XY[c][:, :], in_=PST[c][:, :])
            nc.gpsimd.tensor_tensor(out=SCA[c][:, :], in0=XY[c][:, :],
                                    in1=dws, op=OP.mult)
            nc.sync.dma_start(out=SCA[c][24:29, :], in_=dws[24:29, :])
            mm(ph1[c], "W1", XY[c], True, True)
            mm(pzu[c], "WZY", XY[c], True, False)
            mm(PST[c], "WDXY", XY[c], False, False)
            nc.scalar.activation(out=H1[c][:, :], in_=ph1[c][:, :],
                                 func=AF.Tanh, bias=bias(C_B1 + t))
            mm(ph2[c], "W2", H1[c], True, True)
            nc.scalar.activation(out=H2[c][:, :], in_=ph2[c][:, :],
                                 func=AF.Tanh, bias=bias(C_B2))

        def emit_h2(c, t):
            """MLP back + state update tail."""
            last = (t == t_steps - 1)
            dws = dw_slice(t, c)
            mm(pzu[c], "W3", H2[c], False, True)
            nc.vector.tensor_copy(out=ZUsb[c][:, :], in_=pzu[c][:, :])
            nc.vector.tensor_tensor(out=SCB[c][:, :], in0=ZUsb[c][:, :],
                                    in1=dws, op=OP.mult)
            mm(PST[c], "WDSA", SCA[c], False, False)
            mm(PST[c], "WDZU", ZUsb[c], False, False)
            mm(PST[c], "WDSB", SCB[c], False, last)
            nc.vector.scalar_tensor_tensor(
                out=SCR[c][:, :], in0=ZUsb[c][:, :],
                scalar=float(_ct(t)), in1=ZUsb[c][:, :],
                op0=OP.mult, op1=OP.mult,
                accum_out=lacc[:, 2 * t + c: 2 * t + c + 1])

        emit_dma(0)
        emit_h1(0, 0)
        for t in range(t_steps):
            emit_h2(0, t)
            emit_h1(1, t)
            if t + 1 < t_steps:
                emit_dma(t + 1)
                emit_h1(0, t + 1)
            emit_h2(1, t)

        # ---- final ----
        for c in range(NCH):
            nc.scalar.activation(out=XY[c][:, :], in_=PST[c][:, :],
                                 func=AF.Copy)
            mm(ph1[c], "WE", XY[c], True, True)
            nc.scalar.activation(out=SCR[c][:, :], in_=ph1[c][:, :],
                                 func=AF.Square,
                                 accum_out=eacc[:, c: c + 1])
        nc.sync.dma_start(out=lacc_d[:, :], in_=lacc[:, :])
        nc.sync.dma_start(out=eacc_d[:, :], in_=eacc[:, :])

    nc.compile()
    _BUILT[t_steps] = nc
    return nc


def make_in_maps(inputs):
    bd = pack_weights_bd(inputs)
    ckb = np.concatenate([bd[name] for name in BD_NAMES], axis=0)
    ckf = pack_weights_f32(inputs)
    X0 = np.asarray(inputs["X0"], F32)
    dw = np.asarray(inputs["dw"], F32)
    in_maps = []
    for k in range(NCORES):
        x0p = pack_x0(X0, k)
        in_maps.append({
            "dwat": pack_dwat(dw, k),
            "x0p": np.ascontiguousarray(x0p.reshape(NCH * 128, BK)),
            "ckb": np.ascontiguousarray(ckb),
            "ckf": ckf,
        })
    return in_maps


def kernel(**inputs):
    from concourse.bass_utils import run_bass_kernel_spmd

    in_maps = make_in_maps(inputs)
    nc = build(T)
    res = run_bass_kernel_spmd(nc, in_maps, core_ids=list(range(NCORES)))
    laccs = [r["out_lacc"] for r in res.results]
    eaccs = [r["out_eacc"] for r in res.results]
    return reduce_outputs(laccs, eaccs)


if __name__ == "__main__":
    print("module ok")
